# revision 9
# baseline (speedup 1.0000x reference)
"""Trainium2 Bass kernel for a neural-spline-flow marginal.

Computation: 4-layer MLP conditioner (256->1024->1024->1024->190) over
context [64,1024,256], then 2 rational-quadratic-spline layers transforming
x [64,1024] with a per-batch logdet [64].

Distribution: data-parallel over the batch dim across 8 NeuronCores
(8 batches = 8192 tokens per core); MLP weights replicated per core.

Per-core layout:
  - MLP runs in transposed-activation layout: hT tiles [128 H-partitions,
    8 H-chunks, T tokens] so no inter-layer transposes are needed. Matmul
    inputs are float32r (TF32-like: bf16-rate at moving-dim >= 256 with
    ~1.6e-4 relative error, HW-measured).
  - Context tiles are transposed once on the PE (f32r, via identity).
  - The last layer uses h3T chunks as the *stationary* operand and
    (zero-padded 190->256) Wo as moving, which yields params directly in
    token-major layout [128 tokens, 190].
  - The spline stage is batched as [128, G=16 token-chunks, bins] so every
    vector op amortizes the ~150-cycle DVE fixed cost over 2048 tokens.
    Cumsums use the native tensor_tensor_scan; bucketize-gathers use the
    one-hot trick (difference of monotone `x >= knot` indicators) with the
    three gatherable tensors (knot-x cw, knot-y ch, derivs d) stacked in one
    CQ [128,16,3,33] tile so each gather pass is one mult + one reduce.
"""
import sys
sys.path.insert(0, "/opt/trn_rl_repo")

import numpy as np

import concourse.bacc as bacc
import concourse.tile as tile
from concourse import mybir
from concourse.bass_utils import run_bass_kernel_spmd
from concourse.masks import make_identity

F32 = mybir.dt.float32
F32R = mybir.dt.float32r
ALU = mybir.AluOpType
ACTF = mybir.ActivationFunctionType
AX = mybir.AxisListType

# problem constants
B_TAIL = 4.0
MIN_BW = 1e-3          # min bin width/height
MIN_DER = 1e-3
K = 32                 # num bins
PPL = 95               # params per layer
NPAR = 190

# per-core geometry
N_CORES = 8
TOK = 8192             # tokens per core (8 batches x 1024)
T = 512                # tokens per MLP tile
NT = TOK // T          # 16 MLP tiles
G = 16                 # token-chunks (of 128) per spline megachunk
NMEGA = TOK // (128 * G)   # 4 megachunks
D_IN = 256
H = 1024
KC = H // 128          # 8 k-chunks
WO_PAD = 256           # Wo free dim padded for full-rate f32r matmul

TWO_B_S = 2.0 * B_TAIL * (1.0 - MIN_BW * K)   # scale for normalized cumsum
AFF_STEP = 2.0 * B_TAIL * MIN_BW               # knot affine: aff_j = step*j - B


def _build_core():
    nc = bacc.Bacc(None)

    ctx_d = nc.dram_tensor("context", [TOK, D_IN], F32, kind="ExternalInput")
    x_d = nc.dram_tensor("x", [TOK], F32, kind="ExternalInput")
    w1_d = nc.dram_tensor("W1", [D_IN, H], F32, kind="ExternalInput")
    b1_d = nc.dram_tensor("b1", [H], F32, kind="ExternalInput")
    w2_d = nc.dram_tensor("W2", [H, H], F32, kind="ExternalInput")
    b2_d = nc.dram_tensor("b2", [H], F32, kind="ExternalInput")
    w3_d = nc.dram_tensor("W3", [H, H], F32, kind="ExternalInput")
    b3_d = nc.dram_tensor("b3", [H], F32, kind="ExternalInput")
    wo_d = nc.dram_tensor("Wo", [H, NPAR], F32, kind="ExternalInput")
    bo_d = nc.dram_tensor("bo", [NPAR], F32, kind="ExternalInput")

    y_d = nc.dram_tensor("y", [TOK], F32, kind="ExternalOutput")
    ld_d = nc.dram_tensor("ld", [8], F32, kind="ExternalOutput")

    with tile.TileContext(nc) as tc:
        with (
            tc.tile_pool(name="perm", bufs=1) as perm,
            tc.tile_pool(name="work", bufs=2) as work,
            tc.tile_pool(name="ctp", bufs=1) as ctp,
            tc.tile_pool(name="hbuf", bufs=1) as hbuf,
            tc.tile_pool(name="pbuf", bufs=2) as pbuf,
            tc.tile_pool(name="spl", bufs=1) as spl,
            tc.tile_pool(name="spl2", bufs=1) as spl2,
            tc.tile_pool(name="ps", bufs=2, space="PSUM") as ps,
            tc.tile_pool(name="ps1", bufs=1, space="PSUM") as ps1,
        ):
            # ---------------- persistent setup ----------------
            w1r = perm.tile([128, 2, H], F32R)
            w2r = perm.tile([128, KC, H], F32R)
            w3r = perm.tile([128, KC, H], F32R)
            wor = perm.tile([128, KC, WO_PAD], F32R)
            # zero the 190->256 pad via ACT copies (f32r tiles cannot be
            # memset directly, and f32r matmul inputs need rounding producers)
            zpad = perm.tile([128, WO_PAD - NPAR], F32)
            nc.vector.memset(zpad[:], 0.0)
            for k in range(KC):
                nc.scalar.copy(wor[:, k, NPAR:], zpad[:])

            def load_weight(dram, dst, kchunks, ncols):
                src = dram.rearrange("(k p) m -> p k m", p=128)
                for k in range(kchunks):
                    st = work.tile([128, H], F32, tag="cin")
                    nc.gpsimd.dma_start(st[:, :ncols], src[:, k, :])
                    nc.scalar.copy(dst[:, k, :ncols], st[:, :ncols])

            load_weight(w1_d, w1r, 2, H)
            load_weight(w2_d, w2r, KC, H)
            load_weight(w3_d, w3r, KC, H)
            load_weight(wo_d, wor, KC, NPAR)

            b1s = perm.tile([128, KC], F32)
            b2s = perm.tile([128, KC], F32)
            b3s = perm.tile([128, KC], F32)
            nc.gpsimd.dma_start(b1s[:], b1_d.rearrange("(m p) -> p m", p=128))
            nc.gpsimd.dma_start(b2s[:], b2_d.rearrange("(m p) -> p m", p=128))
            nc.gpsimd.dma_start(b3s[:], b3_d.rearrange("(m p) -> p m", p=128))
            bos = perm.tile([128, NPAR], F32)
            nc.gpsimd.dma_start(bos[:], bo_d[None, :].to_broadcast([128, NPAR]))

            ident = perm.tile([128, 128], F32)
            make_identity(nc, ident[:])
            # seed PE's view of the identity so later transposes carry <=1 wait
            seed_ps = ps1.tile([128, 128], F32, tag="seed")
            nc.tensor.transpose(seed_ps[:], ident[:], ident[:])

            # knot affine row: aff[j] = AFF_STEP*j - B, j=0..32
            aff_i = perm.tile([128, K + 1], mybir.dt.int32)
            nc.gpsimd.iota(aff_i[:], pattern=[[1, K + 1]], base=0, channel_multiplier=0)
            aff = perm.tile([128, K + 1], F32)
            nc.scalar.activation(aff[:], aff_i[:], ACTF.Copy, bias=-B_TAIL, scale=AFF_STEP)

            xq = perm.tile([128, TOK // 128], F32)
            nc.gpsimd.dma_start(xq[:], x_d.rearrange("(c p) -> p c", p=128))
            ldacc = perm.tile([128, TOK // 128], F32)
            nc.vector.memset(ldacc[:], 0.0)
            ones = perm.tile([128, 1], F32)
            nc.vector.memset(ones[:], 1.0)

            y_view = y_d.rearrange("(c p) -> p c", p=128)

            # ---------------- spline stage ----------------
            def spline(params, x_in, y_out, ld_slice, li):
                """params [128,G,190] f32, x_in/y_out [128,G], ld_slice [128,G] view."""
                ofs = li * PPL
                uw = params[:, :, ofs:ofs + K]
                uh = params[:, :, ofs + K:ofs + 2 * K]
                ud = params[:, :, ofs + 2 * K:ofs + 3 * K - 1]

                def bc(t2):   # [128,G] -> [128,G,K]
                    return t2[:, :, None].to_broadcast([128, G, K])

                wE = spl2.tile([128, G, K], F32, tag="wE")
                hE = spl2.tile([128, G, K], F32, tag="hE")
                nc.scalar.activation(wE[:], uw, ACTF.Exp)
                nc.scalar.activation(hE[:], uh, ACTF.Exp)

                CQ = spl2.tile([128, G, 3, K + 1], F32, tag="CQ")
                for g in range(G):
                    nc.vector.tensor_tensor_scan(
                        out=CQ[:, g, 0, 1:], data0=wE[:, g, :], data1=wE[:, g, :],
                        initial=0.0, op0=ALU.add, op1=ALU.bypass)
                for g in range(G):
                    nc.vector.tensor_tensor_scan(
                        out=CQ[:, g, 1, 1:], data0=hE[:, g, :], data1=hE[:, g, :],
                        initial=0.0, op0=ALU.add, op1=ALU.bypass)

                # derivatives: d = [1, softplus(ud)+MIN_DER, 1]
                # softplus(x) = ln(exp(x) + 1); |ud| is small so exp is safe,
                # and this build's ACT tables lack a native Softplus
                eu = spl2.tile([128, G, K - 1], F32, tag="hE", name="eu")
                nc.scalar.activation(eu[:], ud, ACTF.Exp)
                nc.scalar.activation(CQ[:, :, 2, 1:K], eu[:], ACTF.Ln, bias=1.0)
                nc.vector.tensor_scalar_add(CQ[:, :, 2, 1:K], CQ[:, :, 2, 1:K], MIN_DER)
                nc.vector.memset(CQ[:, :, 2, 0:1], 1.0)
                nc.vector.memset(CQ[:, :, 2, K:K + 1], 1.0)

                # normalize cumsums into knot coords: cw = cs*(2Bs/S) + aff
                rsw = spl2.tile([128, G], F32, tag="rsw")
                rsh = spl2.tile([128, G], F32, tag="rsh")
                nc.vector.reciprocal(rsw[:], CQ[:, :, 0, K])
                nc.vector.reciprocal(rsh[:], CQ[:, :, 1, K])
                nc.vector.tensor_scalar_mul(rsw[:], rsw[:], TWO_B_S)
                nc.vector.tensor_scalar_mul(rsh[:], rsh[:], TWO_B_S)
                affb = aff[:, None, 1:].to_broadcast([128, G, K])
                nc.vector.tensor_tensor(CQ[:, :, 0, 1:], CQ[:, :, 0, 1:], bc(rsw), ALU.mult)
                nc.vector.tensor_tensor(CQ[:, :, 0, 1:], CQ[:, :, 0, 1:], affb, ALU.add)
                nc.vector.tensor_tensor(CQ[:, :, 1, 1:], CQ[:, :, 1, 1:], bc(rsh), ALU.mult)
                nc.vector.tensor_tensor(CQ[:, :, 1, 1:], CQ[:, :, 1, 1:], affb, ALU.add)
                nc.vector.memset(CQ[:, :, 0:2, 0], -B_TAIL)

                # bucketize: ge_j = (cw_j <= xc); onehot = ge_j - ge_{j+1}
                xc = spl2.tile([128, G], F32, tag="xc")
                nc.vector.tensor_scalar(
                    out=xc[:], in0=x_in, scalar1=B_TAIL, scalar2=-B_TAIL,
                    op0=ALU.min, op1=ALU.max)
                ge = spl2.tile([128, G, K], F32, tag="ge")
                nc.vector.tensor_tensor(ge[:], CQ[:, :, 0, 0:K], bc(xc), ALU.is_le)
                oh = spl2.tile([128, G, K], F32, tag="oh")
                nc.vector.tensor_tensor(oh[:, :, 0:K - 1], ge[:, :, 0:K - 1], ge[:, :, 1:K], ALU.subtract)
                nc.vector.tensor_copy(oh[:, :, K - 1:K], ge[:, :, K - 1:K])

                # gathers: G0 = {cw,ch,d}[idx], G1 = {cw,ch,d}[idx+1]
                ohb = oh[:, :, None, :].to_broadcast([128, G, 3, K])
                prod = spl2.tile([128, G, 3, K], F32, tag="prod")
                G0 = spl2.tile([128, G, 3], F32, tag="G0")
                G1 = spl2.tile([128, G, 3], F32, tag="G1")
                nc.vector.tensor_tensor(prod[:], CQ[:, :, :, 0:K], ohb, ALU.mult)
                nc.vector.tensor_reduce(G0[:], prod[:], axis=AX.X, op=ALU.add)
                prod2 = spl2.tile([128, G, 3, K], F32, tag="prod")
                nc.vector.tensor_tensor(prod2[:], CQ[:, :, :, 1:], ohb, ALU.mult)
                nc.vector.tensor_reduce(G1[:], prod2[:], axis=AX.X, op=ALU.add)

                in_cw, in_ch, d0 = G0[:, :, 0], G0[:, :, 1], G0[:, :, 2]
                d1 = G1[:, :, 2]

                def t2(tag):
                    return spl2.tile([128, G], F32, tag=tag, name=tag)

                in_wb = t2("in_wb"); in_hb = t2("in_hb")
                nc.vector.tensor_tensor(in_wb[:], G1[:, :, 0], in_cw, ALU.subtract)
                nc.vector.tensor_tensor(in_hb[:], G1[:, :, 1], in_ch, ALU.subtract)
                rwb = t2("rwb")
                nc.vector.reciprocal(rwb[:], in_wb[:])
                theta = t2("theta"); delta = t2("delta")
                nc.vector.tensor_tensor(theta[:], xc[:], in_cw, ALU.subtract)
                nc.vector.tensor_tensor(theta[:], theta[:], rwb[:], ALU.mult)
                nc.vector.tensor_tensor(delta[:], in_hb[:], rwb[:], ALU.mult)
                omt = t2("omt")
                nc.scalar.activation(omt[:], theta[:], ACTF.Copy, bias=1.0, scale=-1.0)
                t1m = t2("t1m"); th2 = t2("th2"); om2 = t2("om2")
                nc.vector.tensor_tensor(t1m[:], theta[:], omt[:], ALU.mult)
                nc.vector.tensor_tensor(th2[:], theta[:], theta[:], ALU.mult)
                nc.vector.tensor_tensor(om2[:], omt[:], omt[:], ALU.mult)

                # den = delta + (d0 + d1 - 2 delta) * t1m
                dpd = t2("dpd")
                nc.vector.tensor_tensor(dpd[:], d0, d1, ALU.add)
                m2d = t2("m2d")
                nc.vector.scalar_tensor_tensor(
                    out=m2d[:], in0=delta[:], scalar=-2.0, in1=dpd[:],
                    op0=ALU.mult, op1=ALU.add)
                den = t2("den")
                nc.vector.tensor_tensor(den[:], m2d[:], t1m[:], ALU.mult)
                nc.vector.tensor_tensor(den[:], den[:], delta[:], ALU.add)
                rden = t2("rden")
                nc.vector.reciprocal(rden[:], den[:])

                # y = in_ch + in_hb*(delta th2 + d0 t1m) * rden
                num = t2("num"); tmp = t2("tmp")
                nc.vector.tensor_tensor(num[:], delta[:], th2[:], ALU.mult)
                nc.vector.tensor_tensor(tmp[:], d0, t1m[:], ALU.mult)
                nc.vector.tensor_tensor(num[:], num[:], tmp[:], ALU.add)
                nc.vector.tensor_tensor(num[:], num[:], in_hb[:], ALU.mult)
                nc.vector.tensor_tensor(num[:], num[:], rden[:], ALU.mult)
                ysp = t2("ysp")
                nc.vector.tensor_tensor(ysp[:], num[:], in_ch, ALU.add)

                # ld = ln(delta^2 (d1 th2 + 2 delta t1m + d0 om2)) - 2 ln(den)
                dn = t2("dn")
                nc.vector.tensor_tensor(dn[:], d1, th2[:], ALU.mult)
                nc.vector.scalar_tensor_tensor(
                    out=tmp[:], in0=delta[:], scalar=2.0, in1=t1m[:],
                    op0=ALU.mult, op1=ALU.mult)
                nc.vector.tensor_tensor(dn[:], dn[:], tmp[:], ALU.add)
                nc.vector.tensor_tensor(tmp[:], d0, om2[:], ALU.mult)
                nc.vector.tensor_tensor(dn[:], dn[:], tmp[:], ALU.add)
                nc.vector.tensor_tensor(tmp[:], delta[:], delta[:], ALU.mult)
                nc.vector.tensor_tensor(dn[:], dn[:], tmp[:], ALU.mult)
                lnd = t2("lnd"); lnden = t2("lnden")
                nc.scalar.activation(lnd[:], dn[:], ACTF.Ln)
                nc.scalar.activation(lnden[:], den[:], ACTF.Ln)
                ld = t2("ld")
                nc.vector.scalar_tensor_tensor(
                    out=ld[:], in0=lnden[:], scalar=-2.0, in1=lnd[:],
                    op0=ALU.mult, op1=ALU.add)

                # outside-tail handling
                ab = t2("ab")
                nc.scalar.activation(ab[:], x_in, ACTF.Abs)
                ins01 = t2("ins01")
                nc.vector.tensor_scalar(out=ins01[:], in0=ab[:], scalar1=B_TAIL,
                                        scalar2=None, op0=ALU.is_le)
                insu8 = spl2.tile([128, G], mybir.dt.uint8, tag="insu8", name="insu8")
                nc.vector.tensor_scalar(out=insu8[:], in0=ab[:], scalar1=B_TAIL,
                                        scalar2=None, op0=ALU.is_le)
                nc.vector.select(y_out[:], insu8[:], ysp[:], x_in)
                nc.vector.tensor_tensor(ld[:], ld[:], ins01[:], ALU.mult)
                nc.vector.tensor_tensor(ld_slice, ld_slice, ld[:], ALU.add)

            # ---------------- main loop ----------------
            for mega in range(NMEGA):
                params = pbuf.tile([128, G, NPAR], F32, tag="params")
                for tt in range(NT // NMEGA):
                    t_idx = mega * (NT // NMEGA) + tt
                    tok0 = t_idx * T
                    # load context tile; transpose on PE (f32), round to f32r
                    # inside the PSUM->SBUF copy-out
                    cin = work.tile([128, 4, D_IN], F32, tag="cin")
                    nc.gpsimd.dma_start(
                        cin[:], ctx_d[tok0:tok0 + T, :].rearrange("(g p) d -> p g d", p=128))
                    ctxT = ctp.tile([128, 2, T], F32R, tag="ctxT")
                    for g in range(4):
                        for k in range(2):
                            tp = ps.tile([128, 128], F32, tag="tp")
                            nc.tensor.transpose(tp[:], cin[:, g, 128 * k:128 * (k + 1)], ident[:])
                            if (g + k) % 2 == 0:
                                nc.vector.tensor_copy(ctxT[:, k, 128 * g:128 * (g + 1)], tp[:])
                            else:
                                nc.scalar.copy(ctxT[:, k, 128 * g:128 * (g + 1)], tp[:])

                    def layer(src, n_k, wts, bias, dst):
                        for m in range(KC):
                            acc = ps.tile([128, T], F32, tag="mm")
                            for k in range(n_k):
                                nc.tensor.matmul(
                                    acc[:], wts[:, k, 128 * m:128 * (m + 1)], src[:, k, :],
                                    start=(k == 0), stop=(k == n_k - 1))
                            if m % 2 == 0:
                                nc.scalar.activation(
                                    dst[:, m, :], acc[:], ACTF.Relu, bias=bias[:, m:m + 1])
                            else:
                                nc.vector.tensor_scalar(
                                    out=dst[:, m, :], in0=acc[:], scalar1=bias[:, m:m + 1],
                                    scalar2=0.0, op0=ALU.add, op1=ALU.max)

                    h1 = hbuf.tile([128, KC, T], F32R, tag="h1")
                    h2 = hbuf.tile([128, KC, T], F32R, tag="h2")
                    h3 = hbuf.tile([128, KC, T], F32R, tag="h3")
                    layer(ctxT, 2, w1r, b1s, h1)
                    layer(h1, KC, w2r, b2s, h2)
                    layer(h2, KC, w3r, b3s, h3)

                    # output layer: params token-major
                    for ts_ in range(4):
                        po = ps.tile([128, WO_PAD], F32, tag="lo")
                        for k in range(KC):
                            nc.tensor.matmul(
                                po[:], h3[:, k, 128 * ts_:128 * (ts_ + 1)], wor[:, k, :],
                                start=(k == 0), stop=(k == KC - 1))
                        cidx = tt * 4 + ts_
                        nc.vector.tensor_tensor(
                            params[:, cidx, :], po[:, 0:NPAR], bos[:], ALU.add)

                sl = slice(G * mega, G * (mega + 1))
                y0 = spl2.tile([128, G], F32, tag="y0")
                y1 = spl2.tile([128, G], F32, tag="y1")
                spline(params, xq[:, sl], y0, ldacc[:, sl], 0)
                spline(params, y0[:], y1, ldacc[:, sl], 1)
                nc.gpsimd.dma_start(y_view[:, sl], y1[:])

            # ---------------- logdet reduction ----------------
            ldp = ps1.tile([1, TOK // 128], F32, tag="ldp")
            nc.tensor.matmul(ldp[:], ones[:], ldacc[:], start=True, stop=True)
            lds = spl.tile([1, TOK // 128], F32)
            nc.vector.tensor_copy(lds[:], ldp[:])
            ld8 = spl.tile([1, 8], F32)
            nc.vector.tensor_reduce(
                ld8[:], lds[:].rearrange("p (b c) -> p b c", b=8),
                axis=AX.X, op=ALU.add)
            nc.gpsimd.dma_start(ld_d[None, :], ld8[:])

    nc.compile()
    return nc


_NC_CACHE = None


def _get_nc():
    global _NC_CACHE
    if _NC_CACHE is None:
        _NC_CACHE = _build_core()
    return _NC_CACHE


def kernel(context, x, W1, b1, W2, b2, W3, b3, Wo, bo):
    context = np.ascontiguousarray(context, dtype=np.float32)
    x = np.ascontiguousarray(x, dtype=np.float32)
    Bsz, N, D = context.shape
    per = Bsz // N_CORES

    nc = _get_nc()
    shared = {
        "W1": np.ascontiguousarray(W1, np.float32),
        "b1": np.ascontiguousarray(b1, np.float32),
        "W2": np.ascontiguousarray(W2, np.float32),
        "b2": np.ascontiguousarray(b2, np.float32),
        "W3": np.ascontiguousarray(W3, np.float32),
        "b3": np.ascontiguousarray(b3, np.float32),
        "Wo": np.ascontiguousarray(Wo, np.float32),
        "bo": np.ascontiguousarray(bo, np.float32),
    }
    in_maps = []
    for c in range(N_CORES):
        in_maps.append({
            "context": context[c * per:(c + 1) * per].reshape(TOK, D_IN),
            "x": x[c * per:(c + 1) * per].reshape(TOK),
            **shared,
        })
    res = run_bass_kernel_spmd(nc, in_maps, list(range(N_CORES))).results

    y = np.concatenate([r["y"].reshape(per, N) for r in res], axis=0)
    ld = np.concatenate([r["ld"].reshape(per) for r in res], axis=0)
    return y.astype(np.float32), ld.astype(np.float32)


# revision 24
# speedup vs baseline: 3101.6885x; 3101.6885x over previous
"""Trainium2 Bass kernel for a neural-spline-flow marginal.

Computation: 4-layer MLP conditioner (256->1024->1024->1024->190) over
context [64,1024,256], then 2 rational-quadratic-spline layers transforming
x [64,1024] with a per-batch logdet [64].

Distribution: data-parallel over the batch dim across 8 NeuronCores
(8 batches = 8192 tokens per core); MLP weights replicated per core.

Per-core layout:
  - MLP runs in transposed-activation layout: hT tiles [128 H-partitions,
    8 H-chunks, T tokens] so no inter-layer transposes are needed. Matmul
    inputs are float32r (TF32-like: bf16-rate at moving-dim >= 256 with
    ~1.6e-4 relative error, HW-measured).
  - Context tiles are transposed once on the PE (f32r, via identity).
  - The last layer uses h3T chunks as the *stationary* operand and
    (zero-padded 190->256) Wo as moving, which yields params directly in
    token-major layout [128 tokens, 190].
  - The spline stage is batched as [128, G=16 token-chunks, bins] so every
    vector op amortizes the ~150-cycle DVE fixed cost over 2048 tokens.
    Cumsums use the native tensor_tensor_scan; bucketize-gathers use the
    one-hot trick (difference of monotone `x >= knot` indicators) with the
    three gatherable tensors (knot-x cw, knot-y ch, derivs d) stacked in one
    CQ [128,16,3,33] tile so each gather pass is one mult + one reduce.
"""
import sys
sys.path.insert(0, "/opt/trn_rl_repo")

import numpy as np

import concourse.bacc as bacc
import concourse.tile as tile
from concourse import mybir
from concourse.bass_utils import run_bass_kernel_spmd
from concourse.masks import make_identity

F32 = mybir.dt.float32
F32R = mybir.dt.float32r
ALU = mybir.AluOpType
ACTF = mybir.ActivationFunctionType
AX = mybir.AxisListType

# problem constants
B_TAIL = 4.0
MIN_BW = 1e-3          # min bin width/height
MIN_DER = 1e-3
K = 32                 # num bins
PPL = 95               # params per layer
NPAR = 190

# per-core geometry
N_CORES = 8
TOK = 8192             # tokens per core (8 batches x 1024)
T = 512                # tokens per MLP tile
NT = TOK // T          # 16 MLP tiles
G = 16                 # token-chunks (of 128) per spline megachunk
NMEGA = TOK // (128 * G)   # 4 megachunks
D_IN = 256
H = 1024
KC = H // 128          # 8 k-chunks
WO_PAD = 256           # Wo free dim padded for full-rate f32r matmul

TWO_B_S = 2.0 * B_TAIL * (1.0 - MIN_BW * K)   # scale for normalized cumsum
AFF_STEP = 2.0 * B_TAIL * MIN_BW               # knot affine: aff_j = step*j - B


def _build_core(bufs_ctxT=2, bufs_mm=2, bufs_cin=3, skip_spline=False, skip_mlp=False,
                repeat=1, spl_gp=True, bufs_mm3=2, demote_w=False):
    nc = bacc.Bacc(None)

    ctx_d = nc.dram_tensor("context", [TOK, D_IN], F32, kind="ExternalInput")
    x_d = nc.dram_tensor("x", [TOK], F32, kind="ExternalInput")
    w1_d = nc.dram_tensor("W1", [D_IN, H], F32, kind="ExternalInput")
    b1_d = nc.dram_tensor("b1", [H], F32, kind="ExternalInput")
    w2_d = nc.dram_tensor("W2", [H, H], F32, kind="ExternalInput")
    b2_d = nc.dram_tensor("b2", [H], F32, kind="ExternalInput")
    w3_d = nc.dram_tensor("W3", [H, H], F32, kind="ExternalInput")
    b3_d = nc.dram_tensor("b3", [H], F32, kind="ExternalInput")
    wo_d = nc.dram_tensor("Wo", [H, NPAR], F32, kind="ExternalInput")
    bo_d = nc.dram_tensor("bo", [NPAR], F32, kind="ExternalInput")

    y_d = nc.dram_tensor("y", [TOK], F32, kind="ExternalOutput")
    ld_d = nc.dram_tensor("ld", [8], F32, kind="ExternalOutput")

    with tile.TileContext(nc) as tc:
        with (
            tc.tile_pool(name="perm", bufs=1) as perm,
            tc.tile_pool(name="work", bufs=bufs_cin) as work,
            tc.tile_pool(name="ctp", bufs=bufs_ctxT) as ctp,
            tc.tile_pool(name="hbuf", bufs=1) as hbuf,
            tc.tile_pool(name="pbuf", bufs=2) as pbuf,
            tc.tile_pool(name="spl", bufs=1) as spl,
            tc.tile_pool(name="spl2", bufs=1) as spl2,
            tc.tile_pool(name="ps", bufs=bufs_mm, space="PSUM") as ps,
            tc.tile_pool(name="ps1", bufs=1, space="PSUM") as ps1,
        ):
            # ---------------- persistent setup ----------------
            w1r = perm.tile([128, 2, H], F32R)
            w2r = perm.tile([128, KC, H], F32R)
            w3r = perm.tile([128, KC, H], F32R)
            wor = perm.tile([128, KC, WO_PAD], F32R)
            # zero the 190->256 pad via ACT copies (f32r tiles cannot be
            # memset directly, and f32r matmul inputs need rounding producers)
            zpad = perm.tile([128, WO_PAD - NPAR], F32)
            nc.vector.memset(zpad[:], 0.0)
            for k in range(KC):
                nc.scalar.copy(wor[:, k, NPAR:], zpad[:])

            def load_weight(dram, dst, kchunks, ncols):
                src = dram.rearrange("(k p) m -> p k m", p=128)
                half = ncols // 2
                for k in range(kchunks):
                    for hh in range(2):
                        cs = slice(hh * half, ncols if hh else half)
                        st = work.tile([128, (H + 1) // 2], F32, tag="wstage", name="st")
                        n = cs.stop - cs.start
                        nc.sync.dma_start(st[:, :n], src[:, k, cs])
                        if (2 * k + hh) % 2 == 0:
                            nc.scalar.copy(dst[:, k, cs], st[:, :n])
                        else:
                            nc.vector.tensor_copy(dst[:, k, cs], st[:, :n])

            load_weight(w1_d, w1r, 2, H)
            if demote_w:
                _svw = tc.cur_priority
                tc.cur_priority = _svw + 120
            load_weight(w2_d, w2r, KC, H)
            load_weight(w3_d, w3r, KC, H)
            load_weight(wo_d, wor, KC, NPAR)
            if demote_w:
                tc.cur_priority = _svw

            b1s = perm.tile([128, KC], F32)
            b2s = perm.tile([128, KC], F32)
            b3s = perm.tile([128, KC], F32)
            nc.gpsimd.dma_start(b1s[:], b1_d.rearrange("(m p) -> p m", p=128))
            nc.gpsimd.dma_start(b2s[:], b2_d.rearrange("(m p) -> p m", p=128))
            nc.gpsimd.dma_start(b3s[:], b3_d.rearrange("(m p) -> p m", p=128))
            bos = perm.tile([128, NPAR], F32)
            nc.gpsimd.dma_start(bos[:], bo_d[None, :].to_broadcast([128, NPAR]))

            with tc.high_priority():
                ident = perm.tile([128, 128], F32)
                make_identity(nc, ident[:])
                # seed PE's view of the identity so later transposes carry <=1 wait
                seed_ps = ps.tile([128, 128], F32, tag="tp", name="seed_ps")
                nc.tensor.transpose(seed_ps[:], ident[:], ident[:])

            # knot affine row: aff[j] = AFF_STEP*j - B, j=0..32
            aff_i = perm.tile([128, K + 1], mybir.dt.int32)
            nc.gpsimd.iota(aff_i[:], pattern=[[1, K + 1]], base=0, channel_multiplier=0)
            aff = perm.tile([128, K + 1], F32)
            nc.scalar.activation(aff[:], aff_i[:], ACTF.Copy, bias=-B_TAIL, scale=AFF_STEP)

            with tc.high_priority():
                xq = perm.tile([128, TOK // 128], F32)
                nc.gpsimd.dma_start(xq[:], x_d.rearrange("(c p) -> p c", p=128))
            ldacc = perm.tile([128, TOK // 128], F32)
            nc.vector.memset(ldacc[:], 0.0)
            ones = perm.tile([128, 1], F32)
            nc.vector.memset(ones[:], 1.0)
            cbt = perm.tile([128, 1], F32)
            nc.vector.memset(cbt[:], B_TAIL)
            cm2 = perm.tile([128, 1], F32)
            nc.vector.memset(cm2[:], -2.0)
            c2 = perm.tile([128, 1], F32)
            nc.vector.memset(c2[:], 2.0)

            y_view = y_d.rearrange("(c p) -> p c", p=128)

            # ---------------- spline stage ----------------
            def spline(params, x_in, y_out, ld_slice, li, Gp=G):
                """params [128,G,190] f32, x_in/y_out [128,G], ld_slice [128,G] view."""
                ofs = li * PPL
                uw = params[:, :, ofs:ofs + K]
                uh = params[:, :, ofs + K:ofs + 2 * K]
                ud = params[:, :, ofs + 2 * K:ofs + 3 * K - 1]

                def bc(t2):   # [128,G] -> [128,G,K]
                    return t2[:, :, None].to_broadcast([128, Gp, K])

                wE = spl2.tile([128, Gp, K], F32, tag="wE")
                hE = spl2.tile([128, Gp, K], F32, tag="hE")
                nc.scalar.activation(wE[:], uw, ACTF.Exp)
                nc.scalar.activation(hE[:], uh, ACTF.Exp)

                CQ = spl2.tile([128, Gp, 3, K + 1], F32, tag="CQ")
                scan_eng = nc.vector
                for g in range(Gp):
                    scan_eng.tensor_tensor_scan(
                        out=CQ[:, g, 0, 1:], data0=wE[:, g, :], data1=wE[:, g, :],
                        initial=0.0, op0=ALU.add, op1=ALU.bypass)
                for g in range(Gp):
                    scan_eng.tensor_tensor_scan(
                        out=CQ[:, g, 1, 1:], data0=hE[:, g, :], data1=hE[:, g, :],
                        initial=0.0, op0=ALU.add, op1=ALU.bypass)

                # derivatives: d = [1, softplus(ud)+MIN_DER, 1]
                # softplus(x) = ln(exp(x) + 1); |ud| is small so exp is safe,
                # and this build's ACT tables lack a native Softplus
                eu = spl2.tile([128, Gp, K - 1], F32, tag="hE", name="eu")
                nc.scalar.activation(eu[:], ud, ACTF.Exp)
                nc.scalar.activation(CQ[:, :, 2, 1:K], eu[:], ACTF.Ln, bias=1.0)
                nc.vector.tensor_scalar_add(CQ[:, :, 2, 1:K], CQ[:, :, 2, 1:K], MIN_DER)
                nc.vector.memset(CQ[:, :, 2, 0:1], 1.0)
                nc.vector.memset(CQ[:, :, 2, K:K + 1], 1.0)

                # normalize cumsums into knot coords: cw = cs*(2Bs/S) + aff
                rsw = spl2.tile([128, Gp], F32, tag="rsw")
                rsh = spl2.tile([128, Gp], F32, tag="rsh")
                nc.vector.reciprocal(rsw[:], CQ[:, :, 0, K])
                nc.vector.reciprocal(rsh[:], CQ[:, :, 1, K])
                nc.vector.tensor_scalar_mul(rsw[:], rsw[:], TWO_B_S)
                nc.vector.tensor_scalar_mul(rsh[:], rsh[:], TWO_B_S)
                affb = aff[:, None, 1:].to_broadcast([128, Gp, K])
                nc.vector.tensor_tensor(CQ[:, :, 0, 1:], CQ[:, :, 0, 1:], bc(rsw), ALU.mult)
                nc.vector.tensor_tensor(CQ[:, :, 0, 1:], CQ[:, :, 0, 1:], affb, ALU.add)
                nc.vector.tensor_tensor(CQ[:, :, 1, 1:], CQ[:, :, 1, 1:], bc(rsh), ALU.mult)
                nc.vector.tensor_tensor(CQ[:, :, 1, 1:], CQ[:, :, 1, 1:], affb, ALU.add)
                nc.vector.memset(CQ[:, :, 0:2, 0], -B_TAIL)

                # bucketize: ge_j = (cw_j <= xc); onehot = ge_j - ge_{j+1}
                xc = spl2.tile([128, Gp], F32, tag="xc")
                nc.vector.tensor_scalar(
                    out=xc[:], in0=x_in, scalar1=B_TAIL, scalar2=-B_TAIL,
                    op0=ALU.min, op1=ALU.max)
                ge = spl2.tile([128, Gp, K], F32, tag="ge")
                nc.vector.tensor_tensor(ge[:], CQ[:, :, 0, 0:K], bc(xc), ALU.is_le)
                oh = spl2.tile([128, Gp, K], F32, tag="oh")
                nc.vector.tensor_tensor(oh[:, :, 0:K - 1], ge[:, :, 0:K - 1], ge[:, :, 1:K], ALU.subtract)
                nc.vector.tensor_copy(oh[:, :, K - 1:K], ge[:, :, K - 1:K])

                # gathers: G0 = {cw,ch,d}[idx], G1 = {cw,ch,d}[idx+1]
                ohb = oh[:, :, None, :].to_broadcast([128, Gp, 3, K])
                prod = spl2.tile([128, Gp, 3, K], F32, tag="prod")
                G0 = spl2.tile([128, Gp, 3], F32, tag="G0")
                G1 = spl2.tile([128, Gp, 3], F32, tag="G1")
                nc.vector.tensor_tensor(prod[:], CQ[:, :, :, 0:K], ohb, ALU.mult)
                nc.vector.tensor_reduce(G0[:], prod[:], axis=AX.X, op=ALU.add)
                prod2 = spl2.tile([128, Gp, 3, K], F32, tag="prod")
                nc.vector.tensor_tensor(prod2[:], CQ[:, :, :, 1:], ohb, ALU.mult)
                nc.vector.tensor_reduce(G1[:], prod2[:], axis=AX.X, op=ALU.add)

                in_cw, in_ch, d0 = G0[:, :, 0], G0[:, :, 1], G0[:, :, 2]
                d1 = G1[:, :, 2]

                def t2(tag):
                    return spl2.tile([128, Gp], F32, tag=tag, name=tag)

                gp = nc.gpsimd if spl_gp else nc.vector
                in_wb = t2("in_wb"); in_hb = t2("in_hb")
                gp.tensor_tensor(in_wb[:], G1[:, :, 0], in_cw, ALU.subtract)
                gp.tensor_tensor(in_hb[:], G1[:, :, 1], in_ch, ALU.subtract)
                rwb = t2("rwb")
                nc.vector.reciprocal(rwb[:], in_wb[:])
                theta = t2("theta"); delta = t2("delta")
                gp.tensor_tensor(theta[:], xc[:], in_cw, ALU.subtract)
                gp.tensor_tensor(theta[:], theta[:], rwb[:], ALU.mult)
                gp.tensor_tensor(delta[:], in_hb[:], rwb[:], ALU.mult)
                omt = t2("omt")
                nc.scalar.activation(omt[:], theta[:], ACTF.Copy, bias=1.0, scale=-1.0)
                t1m = t2("t1m"); th2 = t2("th2"); om2 = t2("om2")
                gp.tensor_tensor(t1m[:], theta[:], omt[:], ALU.mult)
                gp.tensor_tensor(th2[:], theta[:], theta[:], ALU.mult)
                gp.tensor_tensor(om2[:], omt[:], omt[:], ALU.mult)

                # den = delta + (d0 + d1 - 2 delta) * t1m
                dpd = t2("dpd")
                gp.tensor_tensor(dpd[:], d0, d1, ALU.add)
                m2d = t2("m2d")
                gp.tensor_tensor(m2d[:], delta[:], cm2[:].to_broadcast([128, Gp]), ALU.mult)
                gp.tensor_tensor(m2d[:], m2d[:], dpd[:], ALU.add)
                den = t2("den")
                gp.tensor_tensor(den[:], m2d[:], t1m[:], ALU.mult)
                gp.tensor_tensor(den[:], den[:], delta[:], ALU.add)
                rden = t2("rden")
                nc.vector.reciprocal(rden[:], den[:])

                # y = in_ch + in_hb*(delta th2 + d0 t1m) * rden
                num = t2("num"); tmp = t2("tmp")
                gp.tensor_tensor(num[:], delta[:], th2[:], ALU.mult)
                gp.tensor_tensor(tmp[:], d0, t1m[:], ALU.mult)
                gp.tensor_tensor(num[:], num[:], tmp[:], ALU.add)
                gp.tensor_tensor(num[:], num[:], in_hb[:], ALU.mult)
                gp.tensor_tensor(num[:], num[:], rden[:], ALU.mult)
                ysp = t2("ysp")
                gp.tensor_tensor(ysp[:], num[:], in_ch, ALU.add)

                # ld = ln(delta^2 (d1 th2 + 2 delta t1m + d0 om2)) - 2 ln(den)
                dn = t2("dn")
                gp.tensor_tensor(dn[:], d1, th2[:], ALU.mult)
                gp.tensor_tensor(tmp[:], delta[:], c2[:].to_broadcast([128, Gp]), ALU.mult)
                gp.tensor_tensor(tmp[:], tmp[:], t1m[:], ALU.mult)
                gp.tensor_tensor(dn[:], dn[:], tmp[:], ALU.add)
                gp.tensor_tensor(tmp[:], d0, om2[:], ALU.mult)
                gp.tensor_tensor(dn[:], dn[:], tmp[:], ALU.add)
                gp.tensor_tensor(tmp[:], delta[:], delta[:], ALU.mult)
                gp.tensor_tensor(dn[:], dn[:], tmp[:], ALU.mult)
                lnd = t2("lnd"); lnden = t2("lnden")
                nc.scalar.activation(lnd[:], dn[:], ACTF.Ln)
                nc.scalar.activation(lnden[:], den[:], ACTF.Ln)
                ld = t2("ld")
                gp.tensor_tensor(ld[:], lnden[:], cm2[:].to_broadcast([128, Gp]), ALU.mult)
                gp.tensor_tensor(ld[:], ld[:], lnd[:], ALU.add)

                # outside-tail handling
                ab = t2("ab")
                nc.scalar.activation(ab[:], x_in, ACTF.Abs)
                insu8 = spl2.tile([128, Gp], mybir.dt.uint8, tag="insu8", name="insu8")
                nc.vector.tensor_scalar(out=insu8[:], in0=ab[:], scalar1=B_TAIL,
                                        scalar2=None, op0=ALU.is_le)
                ins01 = t2("ins01")
                gp.tensor_copy(ins01[:], insu8[:])
                nc.vector.select(y_out[:], insu8[:], ysp[:], x_in)
                gp.tensor_tensor(ld[:], ld[:], ins01[:], ALU.mult)
                gp.tensor_tensor(ld_slice, ld_slice, ld[:], ALU.add)

            # ---------------- main loop ----------------
            groups = [(4, 16)] * (NMEGA - 1) + [(2, 8), (2, 8)]
            from contextlib import ExitStack as _ES
            _stk = _ES()
            if repeat > 1:
                _stk.enter_context(tc.For_i(0, repeat, 1))
            for rep in range(1):
              t_base = 0
              chunk_base = 0
              for gi, (gtiles, Gp) in enumerate(groups):
                params = pbuf.tile([128, G, NPAR], F32, tag="params", name="params")
                for tt in range(gtiles) if not skip_mlp else []:
                    t_idx = t_base + tt
                    tok0 = t_idx * T
                    # load context tile; transpose on PE (f32), round to f32r
                    # inside the PSUM->SBUF copy-out
                    cin = work.tile([128, 4, D_IN], F32, tag="cin")
                    if rep == 0 and t_idx == 0:
                        with tc.high_priority():
                            nc.sync.dma_start(
                                cin[:], ctx_d[tok0:tok0 + T, :].rearrange("(g p) d -> p g d", p=128))
                    else:
                        nc.sync.dma_start(
                            cin[:], ctx_d[tok0:tok0 + T, :].rearrange("(g p) d -> p g d", p=128))
                    ctxT = ctp.tile([128, 2, T], F32R, tag="ctxT")
                    for g in range(4):
                        for k in range(2):
                            tp = ps.tile([128, 128], F32, tag="tp")
                            nc.tensor.transpose(tp[:], cin[:, g, 128 * k:128 * (k + 1)], ident[:])
                            if (g + k) % 2 == 0:
                                nc.vector.tensor_copy(ctxT[:, k, 128 * g:128 * (g + 1)], tp[:])
                            else:
                                nc.scalar.copy(ctxT[:, k, 128 * g:128 * (g + 1)], tp[:])

                    def layer(src, n_k, wts, bias, dst):
                        for m in range(KC):
                            acc = ps.tile([128, T], F32, tag="mm", bufs=bufs_mm3)
                            for k in range(n_k):
                                nc.tensor.matmul(
                                    acc[:], wts[:, k, 128 * m:128 * (m + 1)], src[:, k, :],
                                    start=(k == 0), stop=(k == n_k - 1))
                            if m % 2 == 0:
                                nc.scalar.activation(
                                    dst[:, m, :], acc[:], ACTF.Relu, bias=bias[:, m:m + 1])
                            else:
                                nc.vector.tensor_scalar(
                                    out=dst[:, m, :], in0=acc[:], scalar1=bias[:, m:m + 1],
                                    scalar2=0.0, op0=ALU.add, op1=ALU.max)

                    h1 = hbuf.tile([128, KC, T], F32R, tag="h1")
                    h2 = hbuf.tile([128, KC, T], F32R, tag="h2")
                    h3 = hbuf.tile([128, KC, T], F32R, tag="h3")
                    layer(ctxT, 2, w1r, b1s, h1)
                    layer(h1, KC, w2r, b2s, h2)
                    layer(h2, KC, w3r, b3s, h3)

                    # output layer: params token-major
                    for ts_ in range(4):
                        po = ps.tile([128, WO_PAD], F32, tag="lo")
                        for k in range(KC):
                            nc.tensor.matmul(
                                po[:], h3[:, k, 128 * ts_:128 * (ts_ + 1)], wor[:, k, :],
                                start=(k == 0), stop=(k == KC - 1))
                        cidx = tt * 4 + ts_
                        nc.vector.tensor_tensor(
                            params[:, cidx, :], po[:, 0:NPAR], bos[:], ALU.add)

                sl = slice(chunk_base, chunk_base + Gp)
                if not skip_spline:
                    y0 = spl2.tile([128, Gp], F32, tag="y0", name="y0")
                    y1 = spl2.tile([128, Gp], F32, tag="y1", name="y1")
                    _sv = tc.cur_priority
                    tc.cur_priority = _sv + 500000
                    spline(params[:, 0:Gp, :], xq[:, sl], y0, ldacc[:, sl], 0, Gp)
                    spline(params[:, 0:Gp, :], y0[:], y1, ldacc[:, sl], 1, Gp)
                    nc.gpsimd.dma_start(y_view[:, sl], y1[:])
                    tc.cur_priority = _sv
                else:
                    nc.gpsimd.dma_start(y_view[:, sl], xq[:, sl])
                t_base += gtiles
                chunk_base += Gp

            _stk.close()
            # ---------------- logdet reduction ----------------
            ldp = ps1.tile([1, TOK // 128], F32, tag="ldp")
            nc.tensor.matmul(ldp[:], ones[:], ldacc[:], start=True, stop=True)
            lds = spl.tile([1, TOK // 128], F32)
            nc.vector.tensor_copy(lds[:], ldp[:])
            ld8 = spl.tile([1, 8], F32)
            nc.vector.tensor_reduce(
                ld8[:], lds[:].rearrange("p (b c) -> p b c", b=8),
                axis=AX.X, op=ALU.add)
            nc.gpsimd.dma_start(ld_d[None, :], ld8[:])

    nc.compile()
    return nc


_NC_CACHE = None


def _get_nc():
    global _NC_CACHE
    if _NC_CACHE is None:
        _NC_CACHE = _build_core()
    return _NC_CACHE


def kernel(context, x, W1, b1, W2, b2, W3, b3, Wo, bo):
    context = np.ascontiguousarray(context, dtype=np.float32)
    x = np.ascontiguousarray(x, dtype=np.float32)
    Bsz, N, D = context.shape
    per = Bsz // N_CORES

    nc = _get_nc()
    shared = {
        "W1": np.ascontiguousarray(W1, np.float32),
        "b1": np.ascontiguousarray(b1, np.float32),
        "W2": np.ascontiguousarray(W2, np.float32),
        "b2": np.ascontiguousarray(b2, np.float32),
        "W3": np.ascontiguousarray(W3, np.float32),
        "b3": np.ascontiguousarray(b3, np.float32),
        "Wo": np.ascontiguousarray(Wo, np.float32),
        "bo": np.ascontiguousarray(bo, np.float32),
    }
    in_maps = []
    for c in range(N_CORES):
        in_maps.append({
            "context": context[c * per:(c + 1) * per].reshape(TOK, D_IN),
            "x": x[c * per:(c + 1) * per].reshape(TOK),
            **shared,
        })
    res = run_bass_kernel_spmd(nc, in_maps, list(range(N_CORES))).results

    y = np.concatenate([r["y"].reshape(per, N) for r in res], axis=0)
    ld = np.concatenate([r["ld"].reshape(per) for r in res], axis=0)
    return y.astype(np.float32), ld.astype(np.float32)
